# revision 8
# baseline (speedup 1.0000x reference)
"""Trainium2 Bass kernel for GUIDEModel (4x GraphConv + 4x edge-softmax GNA layers).

Strategy: edges sorted by dst, dst-range sharded across 8 cores (49 groups of 128
dst nodes per core).  Per layer: shard-local node matmul -> AllGather table ->
per-group indirect gather of src rows + one-hot segment-sum matmul into PSUM.
"""

import math
import os
import sys

import numpy as np

sys.path.insert(0, "/opt/trn_rl_repo")
os.environ.setdefault("MYCRO_LOCAL_CACHE", "1")

import concourse.bass as bass
import concourse.tile as tile
from concourse import bacc, bass_isa, mybir
from concourse.bass_utils import run_bass_kernel_spmd
from concourse.masks import make_identity

F32 = mybir.dt.float32
I32 = mybir.dt.int32

N = 50000
E = 1600000
NCORES = 8
P = 128
NG = 392            # total dst groups of 128 nodes (392*128 = 50176)
GPC = NG // NCORES  # 49 groups per core
RPC = GPC * P       # 6272 rows per core
NPAD = NG * P       # 50176

ATTR_D, ATTR_H = 512, 256
STRUCT_D, STRUCT_H = 6, 32
GCN_DIMS = [(ATTR_D, ATTR_H), (ATTR_H, ATTR_H), (ATTR_H, ATTR_H), (ATTR_H, ATTR_D)]
GNA_DIMS = [(STRUCT_D, STRUCT_H), (STRUCT_H, STRUCT_H), (STRUCT_H, STRUCT_H), (STRUCT_H, STRUCT_D)]

PAD_LDST = 300      # local-dst value for padding edges: never matches iota 0..127

GATHER_MULTI = False  # one indirect DMA per group ([128, T] offsets) vs per tile


def _build_program(T):
    nc = bacc.Bacc("TRN2", target_bir_lowering=False, debug=False, num_devices=NCORES)

    # ---- external inputs ----
    x0 = nc.dram_tensor("x0", [RPC, ATTR_D], F32, kind="ExternalInput")
    xs0 = nc.dram_tensor("xs0", [RPC, STRUCT_D], F32, kind="ExternalInput")
    src_pk = nc.dram_tensor("src_pk", [GPC, P, T], I32, kind="ExternalInput")
    ldst_pk = nc.dram_tensor("ldst_pk", [GPC, P, T], I32, kind="ExternalInput")
    so_pk = nc.dram_tensor("so_pk", [P, GPC], F32, kind="ExternalInput")
    si_pk = nc.dram_tensor("si_pk", [P, GPC], F32, kind="ExternalInput")

    gw, gb = [], []
    for i, (di, do) in enumerate(GCN_DIMS):
        gw.append(nc.dram_tensor(f"gw{i}", [P, di // P, do], F32, kind="ExternalInput"))
        gb.append(nc.dram_tensor(f"gb{i}", [P, do], F32, kind="ExternalInput"))
    swc, sw1, sb2c, sb1 = [], [], [], []
    for i, (di, do) in enumerate(GNA_DIMS):
        swc.append(nc.dram_tensor(f"swc{i}", [di, do + 1], F32, kind="ExternalInput"))
        sw1.append(nc.dram_tensor(f"sw1{i}", [di, do], F32, kind="ExternalInput"))
        sb2c.append(nc.dram_tensor(f"sb2c{i}", [P, do + 1], F32, kind="ExternalInput"))
        sb1.append(nc.dram_tensor(f"sb1{i}", [P, do], F32, kind="ExternalInput"))

    # ---- outputs ----
    attr_out = nc.dram_tensor("attr_out", [RPC, ATTR_D], F32, kind="ExternalOutput")
    struct_out = nc.dram_tensor("struct_out", [RPC, STRUCT_D], F32, kind="ExternalOutput")

    # ---- internals ----
    # GCN: gather tables (post-AllGather, replicated) and shard staging
    g_shard = [nc.dram_tensor(f"g_shard{i}", [RPC, ATTR_H], F32) for i in range(4)]
    g_table = [
        nc.dram_tensor(f"g_table{i}", [NPAD, ATTR_H], F32, addr_space="Shared")
        for i in range(4)
    ]
    x_sh = [None, nc.dram_tensor("x_sh1", [RPC, ATTR_H], F32),
            nc.dram_tensor("x_sh2", [RPC, ATTR_H], F32)]
    # GNA
    s_shard = [nc.dram_tensor(f"s_shard{i}", [RPC, GNA_DIMS[i][1] + 1], F32) for i in range(4)]
    s_table = [
        nc.dram_tensor(f"s_table{i}", [NPAD, GNA_DIMS[i][1] + 1], F32, addr_space="Shared")
        for i in range(4)
    ]
    xs_sh = [None, nc.dram_tensor("xs_sh1", [RPC, STRUCT_H], F32),
             nc.dram_tensor("xs_sh2", [RPC, STRUCT_H], F32),
             nc.dram_tensor("xs_sh3", [RPC, STRUCT_H], F32)]

    groups = [list(range(NCORES))]

    import contextlib

    with tile.TileContext(nc) as tc, contextlib.ExitStack() as ctx:
        const_p = ctx.enter_context(tc.tile_pool(name="const", bufs=1))
        w_p = ctx.enter_context(tc.tile_pool(name="weights", bufs=1))
        mat_p = ctx.enter_context(tc.tile_pool(name="mat", bufs=3))
        gath_p = ctx.enter_context(tc.tile_pool(name="gath", bufs=2))
        sel_p = ctx.enter_context(tc.tile_pool(name="sel", bufs=2))
        y_p = ctx.enter_context(tc.tile_pool(name="y", bufs=3))
        small_p = ctx.enter_context(tc.tile_pool(name="small", bufs=4))
        res_p = ctx.enter_context(tc.tile_pool(name="res", bufs=2))
        ps_seg = ctx.enter_context(tc.tile_pool(name="ps_seg", bufs=2, space="PSUM"))
        ps_tr = ctx.enter_context(tc.tile_pool(name="ps_tr", bufs=2, space="PSUM"))
        ps_mm = ctx.enter_context(tc.tile_pool(name="ps_mm", bufs=1, space="PSUM"))

        ident = const_p.tile([P, P], F32)
        make_identity(nc, ident[:])
        iota_row = const_p.tile([P, 1, P], I32)
        nc.gpsimd.iota(iota_row[:, 0, :], pattern=[[1, P]], channel_multiplier=0)
        so_sb = const_p.tile([P, GPC], F32)
        nc.sync.dma_start(so_sb[:], so_pk[:])
        si_sb = const_p.tile([P, GPC], F32)
        nc.sync.dma_start(si_sb[:], si_pk[:])

        # -------- helper: gather + one-hot segment matmul --------
        def seg_gather(table_ap, width, g, rhs_hook=None):
            """Returns PSUM tile [P, width_out] with segment sums for group g.

            rhs_hook(gath, sel) -> (rhs_ap_3d, width_out) may transform the
            gathered rows before the matmul (used by GNA)."""
            srcs = small_p.tile([P, T], I32)
            nc.sync.dma_start(srcs[:], src_pk[g])
            ldst = small_p.tile([P, T, 1], I32)
            nc.sync.dma_start(ldst[:, :, 0], ldst_pk[g])
            sel = sel_p.tile([P, T, P], F32)
            nc.vector.tensor_tensor(
                out=sel[:],
                in0=ldst[:].to_broadcast([P, T, P]),
                in1=iota_row[:].to_broadcast([P, T, P]),
                op=mybir.AluOpType.is_equal,
            )
            gath = gath_p.tile([P, T, width], F32)
            if GATHER_MULTI:
                nc.gpsimd.indirect_dma_start(
                    out=gath[:],
                    out_offset=None,
                    in_=table_ap,
                    in_offset=bass.IndirectOffsetOnAxis(ap=srcs[:], axis=0),
                )
            else:
                for t in range(T):
                    nc.gpsimd.indirect_dma_start(
                        out=gath[:, t, :],
                        out_offset=None,
                        in_=table_ap,
                        in_offset=bass.IndirectOffsetOnAxis(ap=srcs[:, t : t + 1], axis=0),
                    )
            if rhs_hook is not None:
                rhs, wout = rhs_hook(gath, sel)
            else:
                rhs, wout = gath, width
            seg = ps_seg.tile([P, wout], F32, space="PSUM")
            for t in range(T):
                nc.tensor.matmul(
                    out=seg[:],
                    lhsT=sel[:, t, :],
                    rhs=rhs[:, t, :],
                    start=(t == 0),
                    stop=(t == T - 1),
                )
            return seg

        # -------- helper: rows @ W via per-tile transpose --------
        def tile_matmul_rows(xs_tile, di, w_sb, wout, extra=None):
            """xs_tile [P, di] (rows on partitions) @ W  ->  PSUM [P, wout].

            w_sb view: [P, di//P, wout]. extra: optional second rhs list
            [(w2_sb_ap, wout2)] computed off the same transposes."""
            nk = di // P
            outs = []
            h = ps_mm.tile([P, wout], F32, space="PSUM")
            outs.append(h)
            extras = []
            if extra:
                for w2, wout2 in extra:
                    e = ps_mm.tile([P, wout2], F32, space="PSUM")
                    extras.append((e, w2))
            for k in range(nk):
                xtp = ps_tr.tile([P, P], F32, space="PSUM")
                nc.tensor.transpose(xtp[:], xs_tile[:, k * P : (k + 1) * P], ident[:])
                xT = mat_p.tile([P, P], F32)
                nc.vector.tensor_copy(xT[:], xtp[:])
                nc.tensor.matmul(
                    out=h[:], lhsT=xT[:], rhs=w_sb[:, k, :],
                    start=(k == 0), stop=(k == nk - 1),
                )
                for e, w2 in extras:
                    nc.tensor.matmul(
                        out=e[:], lhsT=xT[:], rhs=w2[:, k, :],
                        start=(k == 0), stop=(k == nk - 1),
                    )
            return outs + [e for e, _ in extras]

        # ==================== GCN layers ====================
        for i in range(4):
            di, do = GCN_DIMS[i]

            # ---- A: shard matmul -> g_shard[i] (layers 0..2); layer 3 table
            #      (pre-scaled x3) was written by layer 2's post phase.
            if i < 3:
                w_sb = w_p.tile([P, di // P, ATTR_H], F32)
                nc.sync.dma_start(w_sb[:], gw[i][:])
                x_dram = x0 if i == 0 else x_sh[i]
                for g in range(GPC):
                    x_t = mat_p.tile([P, di], F32)
                    nc.sync.dma_start(x_t[:], x_dram[g * P : (g + 1) * P, :])
                    xs_t = mat_p.tile([P, di], F32)
                    nc.scalar.activation(
                        xs_t[:], x_t[:], mybir.ActivationFunctionType.Copy,
                        scale=so_sb[:, g : g + 1],
                    )
                    (h_ps,) = tile_matmul_rows(xs_t, di, w_sb, ATTR_H)
                    h_sb = mat_p.tile([P, ATTR_H], F32)
                    nc.vector.tensor_copy(h_sb[:], h_ps[:])
                    nc.sync.dma_start(g_shard[i][g * P : (g + 1) * P, :], h_sb[:])

            # ---- B: AllGather shard -> table
            nc.gpsimd.collective_compute(
                "AllGather",
                mybir.AluOpType.bypass,
                replica_groups=groups,
                ins=[g_shard[i][:]],
                outs=[g_table[i][:]],
            )

            # ---- C: gather-aggregate per group
            b_sb = w_p.tile([P, do], F32)
            nc.sync.dma_start(b_sb[:], gb[i][:])
            if i == 3:
                w3_sb = w_p.tile([P, ATTR_H // P, ATTR_D], F32)
                nc.sync.dma_start(w3_sb[:], gw[3][:])
            for g in range(GPC):
                seg = seg_gather(g_table[i][:], ATTR_H, g)
                if i < 3:
                    t1 = y_p.tile([P, do], F32)
                    nc.scalar.activation(
                        t1[:], seg[:], mybir.ActivationFunctionType.Copy,
                        scale=si_sb[:, g : g + 1],
                    )
                    nc.vector.tensor_tensor(
                        out=t1[:], in0=t1[:], in1=b_sb[:], op=mybir.AluOpType.add
                    )
                    y = y_p.tile([P, do], F32)
                    nc.scalar.activation(y[:], t1[:], mybir.ActivationFunctionType.Relu)
                    if i == 2:
                        # pre-scale next layer's aggregation table by rsqrt(deg_out)
                        nc.scalar.activation(
                            y[:], y[:], mybir.ActivationFunctionType.Copy,
                            scale=so_sb[:, g : g + 1],
                        )
                        nc.sync.dma_start(g_shard[3][g * P : (g + 1) * P, :], y[:])
                    else:
                        nc.sync.dma_start(x_sh[i + 1][g * P : (g + 1) * P, :], y[:])
                else:
                    # z = seg [P, 256]; h = z @ W3 [P, 512]
                    z_sb = mat_p.tile([P, ATTR_H], F32)
                    nc.vector.tensor_copy(z_sb[:], seg[:])
                    (h_ps,) = tile_matmul_rows(z_sb, ATTR_H, w3_sb, ATTR_D)
                    t1 = y_p.tile([P, ATTR_D], F32)
                    nc.scalar.activation(
                        t1[:], h_ps[:], mybir.ActivationFunctionType.Copy,
                        scale=si_sb[:, g : g + 1],
                    )
                    nc.vector.tensor_tensor(
                        out=t1[:], in0=t1[:], in1=b_sb[:], op=mybir.AluOpType.add
                    )
                    y = y_p.tile([P, ATTR_D], F32)
                    nc.scalar.activation(y[:], t1[:], mybir.ActivationFunctionType.Relu)
                    nc.sync.dma_start(attr_out[g * P : (g + 1) * P, :], y[:])

        # ==================== GNA layers ====================
        for i in range(4):
            di, do = GNA_DIMS[i]
            dw = do + 1

            # ---- A: node matmuls -> y2q shard (+ resident y1)
            wc_sb = w_p.tile([di, dw], F32)
            nc.sync.dma_start(wc_sb[:], swc[i][:])
            w1_sb = w_p.tile([di, do], F32)
            nc.sync.dma_start(w1_sb[:], sw1[i][:])
            b2c_sb = w_p.tile([P, dw], F32)
            nc.sync.dma_start(b2c_sb[:], sb2c[i][:])
            b1_sb = w_p.tile([P, do], F32)
            nc.sync.dma_start(b1_sb[:], sb1[i][:])
            y1_res = res_p.tile([P, GPC, do], F32)

            xs_dram = xs0 if i == 0 else xs_sh[i]
            for g in range(GPC):
                x_t = mat_p.tile([P, di], F32)
                nc.sync.dma_start(x_t[:], xs_dram[g * P : (g + 1) * P, :])
                xtp = ps_tr.tile([P, P], F32, space="PSUM")
                nc.tensor.transpose(xtp[:di, :], x_t[:], ident[:])
                xT = mat_p.tile([P, P], F32)
                nc.vector.tensor_copy(xT[:di, :], xtp[:di, :])
                y2q_ps = ps_mm.tile([P, dw], F32, space="PSUM")
                nc.tensor.matmul(
                    out=y2q_ps[:], lhsT=xT[:di, :], rhs=wc_sb[:], start=True, stop=True
                )
                y1_ps = ps_mm.tile([P, do], F32, space="PSUM")
                nc.tensor.matmul(
                    out=y1_ps[:], lhsT=xT[:di, :], rhs=w1_sb[:], start=True, stop=True
                )
                y2q_sb = mat_p.tile([P, dw], F32)
                nc.vector.tensor_tensor(
                    out=y2q_sb[:], in0=y2q_ps[:], in1=b2c_sb[:], op=mybir.AluOpType.add
                )
                nc.vector.tensor_tensor(
                    out=y1_res[:, g, :], in0=y1_ps[:], in1=b1_sb[:], op=mybir.AluOpType.add
                )
                nc.sync.dma_start(s_shard[i][g * P : (g + 1) * P, :], y2q_sb[:])

            # ---- B: AllGather
            nc.gpsimd.collective_compute(
                "AllGather",
                mybir.AluOpType.bypass,
                replica_groups=groups,
                ins=[s_shard[i][:]],
                outs=[s_table[i][:]],
            )

            # ---- C: edge softmax + weighted segment sum
            for g in range(GPC):

                def rhs_hook(gath, sel, do=do, dw=dw):
                    # qs = gath[:, :, do]; ex = exp(-qs + min_group(qs))
                    nqs = small_p.tile([P, T], F32)
                    nc.vector.tensor_scalar(
                        out=nqs[:], in0=gath[:, :, do], scalar1=-1.0, scalar2=None,
                        op0=mybir.AluOpType.mult,
                    )
                    colmax = small_p.tile([P, 1], F32)
                    nc.vector.tensor_reduce(
                        out=colmax[:], in_=nqs[:], axis=mybir.AxisListType.X,
                        op=mybir.AluOpType.max,
                    )
                    smax = small_p.tile([P, 1], F32)
                    nc.gpsimd.partition_all_reduce(
                        smax[:], colmax[:], channels=P, reduce_op=bass_isa.ReduceOp.max
                    )
                    negs = small_p.tile([P, 1], F32)
                    nc.vector.tensor_scalar(
                        out=negs[:], in0=smax[:], scalar1=-1.0, scalar2=None,
                        op0=mybir.AluOpType.mult,
                    )
                    ex = small_p.tile([P, T, 1], F32)
                    nc.scalar.activation(
                        ex[:, :, 0], gath[:, :, do], mybir.ActivationFunctionType.Exp,
                        bias=negs[:], scale=-1.0,
                    )
                    rhs = gath_p.tile([P, T, dw], F32)
                    nc.vector.tensor_tensor(
                        out=rhs[:, :, 0:do],
                        in0=gath[:, :, 0:do],
                        in1=ex[:].to_broadcast([P, T, do]),
                        op=mybir.AluOpType.mult,
                    )
                    nc.vector.tensor_copy(rhs[:, :, do], ex[:, :, 0])
                    return rhs, dw

                seg = seg_gather(s_table[i][:], dw, g, rhs_hook=rhs_hook)
                den = small_p.tile([P, 1], F32)
                nc.vector.tensor_scalar(
                    out=den[:], in0=seg[:, do : do + 1], scalar1=0.0, scalar2=None,
                    op0=mybir.AluOpType.is_equal,
                )
                nc.vector.tensor_tensor(
                    out=den[:], in0=den[:], in1=seg[:, do : do + 1], op=mybir.AluOpType.add
                )
                rec = small_p.tile([P, 1], F32)
                nc.vector.reciprocal(rec[:], den[:])
                res = y_p.tile([P, do], F32)
                nc.vector.tensor_tensor(
                    out=res[:], in0=seg[:, 0:do],
                    in1=rec[:].to_broadcast([P, do]), op=mybir.AluOpType.mult,
                )
                nc.vector.tensor_tensor(
                    out=res[:], in0=res[:], in1=y1_res[:, g, :], op=mybir.AluOpType.add
                )
                y = y_p.tile([P, do], F32)
                nc.scalar.activation(y[:], res[:], mybir.ActivationFunctionType.Relu)
                if i < 3:
                    nc.sync.dma_start(xs_sh[i + 1][g * P : (g + 1) * P, :], y[:])
                else:
                    nc.sync.dma_start(struct_out[g * P : (g + 1) * P, :], y[:])

    nc.compile()
    return nc


def _host_prep(attr_feat, struct_feat, src, dst, params):
    deg_out = np.bincount(src, minlength=N).astype(np.float64)
    deg_in = np.bincount(dst, minlength=N).astype(np.float64)
    so = (1.0 / np.sqrt(np.clip(deg_out, 1.0, None))).astype(np.float32)
    si = (1.0 / np.sqrt(np.clip(deg_in, 1.0, None))).astype(np.float32)

    order = np.argsort(dst, kind="stable")
    ssrc = np.ascontiguousarray(src[order]).astype(np.int32)
    sdst = np.ascontiguousarray(dst[order]).astype(np.int32)
    bounds = np.searchsorted(sdst, np.arange(NG + 1, dtype=np.int64) * P)
    cnts = np.diff(bounds)
    T = int(math.ceil(cnts.max() / P))

    src_pk = np.zeros((NG, P, T), np.int32)
    ldst_pk = np.full((NG, P, T), PAD_LDST, np.int32)
    for g in range(NG):
        b0, b1 = bounds[g], bounds[g + 1]
        cnt = b1 - b0
        if cnt == 0:
            continue
        tmp_s = np.zeros(T * P, np.int32)
        tmp_s[:cnt] = ssrc[b0:b1]
        tmp_d = np.full(T * P, PAD_LDST, np.int32)
        tmp_d[:cnt] = sdst[b0:b1] - g * P
        src_pk[g] = tmp_s.reshape(T, P).T
        ldst_pk[g] = tmp_d.reshape(T, P).T

    def pack_rows(v):  # [N]-vector -> per-core [P, GPC]
        vp = np.ones(NPAD, np.float32)
        vp[:N] = v
        return vp.reshape(NCORES, GPC, P).transpose(0, 2, 1).copy()  # [NC, P, GPC]

    so_pk = pack_rows(so)
    si_pk = pack_rows(si)

    xpad = np.zeros((NPAD, ATTR_D), np.float32)
    xpad[:N] = attr_feat
    spad = np.zeros((NPAD, STRUCT_D), np.float32)
    spad[:N] = struct_feat

    common = {}
    for i, (di, do) in enumerate(GCN_DIMS):
        W = np.asarray(params[f"gcn_W{i}"], np.float32)
        common[f"gw{i}"] = np.ascontiguousarray(
            W.reshape(di // P, P, do).transpose(1, 0, 2)
        )
        common[f"gb{i}"] = np.broadcast_to(
            np.asarray(params[f"gcn_b{i}"], np.float32), (P, do)
        ).copy()
    for i, (di, do) in enumerate(GNA_DIMS):
        W1 = np.asarray(params[f"gna_W1_{i}"], np.float32)  # [do, di]
        W2 = np.asarray(params[f"gna_W2_{i}"], np.float32)  # [do, di]
        b2 = np.asarray(params[f"gna_b2_{i}"], np.float32)
        b1 = np.asarray(params[f"gna_b1_{i}"], np.float32)
        a = np.asarray(params[f"gna_a_{i}"], np.float32)    # [1, do]
        wv = (a @ W2).reshape(di, 1)                         # [di, 1]
        common[f"swc{i}"] = np.ascontiguousarray(np.concatenate([W2.T, wv], axis=1))
        common[f"sw1{i}"] = np.ascontiguousarray(W1.T)
        b2c = np.concatenate([b2, np.float32(np.sum(a * b2)).reshape(1)])
        common[f"sb2c{i}"] = np.broadcast_to(b2c.astype(np.float32), (P, do + 1)).copy()
        common[f"sb1{i}"] = np.broadcast_to(b1, (P, do)).copy()

    in_maps = []
    for c in range(NCORES):
        m = dict(common)
        m["x0"] = np.ascontiguousarray(xpad[c * RPC : (c + 1) * RPC])
        m["xs0"] = np.ascontiguousarray(spad[c * RPC : (c + 1) * RPC])
        m["src_pk"] = np.ascontiguousarray(src_pk[c * GPC : (c + 1) * GPC])
        m["ldst_pk"] = np.ascontiguousarray(ldst_pk[c * GPC : (c + 1) * GPC])
        m["so_pk"] = np.ascontiguousarray(so_pk[c])
        m["si_pk"] = np.ascontiguousarray(si_pk[c])
        in_maps.append(m)
    return T, in_maps


_CACHE = {}


def kernel(attr_feat, struct_feat, src, dst, params):
    attr_feat = np.asarray(attr_feat, np.float32)
    struct_feat = np.asarray(struct_feat, np.float32)
    src = np.asarray(src, np.int32)
    dst = np.asarray(dst, np.int32)
    T, in_maps = _host_prep(attr_feat, struct_feat, src, dst, params)
    if T not in _CACHE:
        _CACHE[T] = _build_program(T)
    nc = _CACHE[T]
    res = run_bass_kernel_spmd(nc, in_maps, list(range(NCORES))).results
    attr = np.concatenate([res[c]["attr_out"] for c in range(NCORES)], axis=0)[:N]
    struct = np.concatenate([res[c]["struct_out"] for c in range(NCORES)], axis=0)[:N]
    return attr, struct
